# revision 1
# baseline (speedup 1.0000x reference)
"""Trainium2 Bass kernel for nn_ContinuousGenHyperConnections.

Sharding: data-parallel over the batch dim B=8192 across 8 NeuronCores
(1024 rows each). All weights replicated; no collectives.

Per-core dataflow (B_loc=1024 -> 8 b-tiles of 128 rows; proj in blocks of 4):
  P1 per tile : DMA x fp32 chunks; cast->bf16 (DVE) into per-tile resident
                x_bf; sum-of-squares via ACT Square accum_out.
  P2 per tile : s = rsqrt(mean(x^2)+eps)                     [128,1] f32
  P3 per block: xT [128d,512b] via strided fp32 DMA straight from x HBM
                (independent of P1), 64-chunk accumulated float32r matmul
                -> proj.T [42,512] (W_cat rows: conv16|diss16|dtc|dtd|
                read4|write4), PE-transpose, scale by s.
  P4 per block: per-row 4x4 generator math batched over 4 tiles: skew,
                K=RR^T, dt sigmoids, A, expm (order-8 Taylor + 4 squarings),
                rw/ww, c = E^T rw.  fp32 on DVE (adds on GpSimd).
  P5 per tile : branch = sum_j c_j x_j (bf16 TS-mult + TT-add),
                16 PE-transposes -> branchT (psum->sbuf copies ACT/DVE).
  P6 per tile : y = branch @ W_mod.T computed directly in [b, e] layout:
                lhsT = branchT chunk (stationary), rhs = W_mod.T chunk
                (moving, resident bf16), N=512 -> y_nb [128,2048] bf16.
  P7 per tile : out_n = sum_j E_nj x_j + ww_n y via TS-mults (DVE) +
                adds (DVE/GpSimd), final add fp32 -> DMA out.
"""

import os
import sys

sys.path.insert(0, "/opt/trn_rl_repo")

import numpy as np
import ml_dtypes

BF16 = ml_dtypes.bfloat16

DT_MIN, DT_MAX = 1e-3, 1.0
EPS = 1e-6
NS = 4  # streams
EMB = 2048
IN_DIM = 8192
N_CORES = 8
NPROJ = 42  # 16 conv + 16 diss + 1 dtc + 1 dtd + 4 read + 4 write


def _build(B_loc, scal, num_devices=N_CORES):
    import concourse.bacc as bacc
    import concourse.mybir as mybir
    import concourse.tile as tile
    from concourse.masks import make_identity
    from contextlib import ExitStack

    dt = mybir.dt
    Alu = mybir.AluOpType
    Act = mybir.ActivationFunctionType
    Axis = mybir.AxisListType

    NT = B_loc // 128
    TPB = min(4, NT)          # tiles per proj block
    NBLK = NT // TPB
    NCH = IN_DIM // 128       # 64 contraction chunks
    NB = TPB * 128            # rows per proj block

    # expm 2^-4 prescale folded into dt: dt_eff = (DT_MIN + range*sig)/16
    R_SIG = (DT_MAX - DT_MIN) / 16.0
    C_SIG = DT_MIN / 16.0

    nc = bacc.Bacc("TRN2", target_bir_lowering=False, debug=False,
                   num_devices=num_devices)

    x_ext = nc.declare_dram_parameter("x", [B_loc, IN_DIM], dt.float32,
                                      isOutput=False)
    wcatT_ext = nc.declare_dram_parameter("wcatT", [128, NCH, NPROJ],
                                          dt.bfloat16, isOutput=False)
    wmodT_ext = nc.declare_dram_parameter("wmodT", [128, 16, EMB],
                                          dt.float8e4, isOutput=False)
    cpack_ext = nc.declare_dram_parameter("cpack", [58], dt.float32,
                                          isOutput=False)
    out_ext = nc.declare_dram_parameter("out", [B_loc, NS, EMB], dt.float32,
                                        isOutput=True)

    with tile.TileContext(nc) as tc, ExitStack() as ctx:
        const_pool = ctx.enter_context(tc.tile_pool(name="const", bufs=1))
        dram_pool = ctx.enter_context(
            tc.tile_pool(name="dram", bufs=1, space="DRAM"))
        p1_pool = ctx.enter_context(tc.tile_pool(name="p1", bufs=2))
        xbb_pool = ctx.enter_context(tc.tile_pool(name="xbb", bufs=5))
        xt_pool = ctx.enter_context(tc.tile_pool(name="xt", bufs=4))
        small_pool = ctx.enter_context(tc.tile_pool(name="small", bufs=2))
        sm1_pool = ctx.enter_context(tc.tile_pool(name="sm1", bufs=1))
        str_pool = ctx.enter_context(tc.tile_pool(name="stream", bufs=2))
        brt_pool = ctx.enter_context(tc.tile_pool(name="brt", bufs=1))
        out_pool = ctx.enter_context(tc.tile_pool(name="outp", bufs=2))
        ps_proj = ctx.enter_context(
            tc.tile_pool(name="ps_proj", bufs=1, space="PSUM"))
        ps_trp = ctx.enter_context(
            tc.tile_pool(name="ps_trp", bufs=1, space="PSUM"))
        ps_br = ctx.enter_context(
            tc.tile_pool(name="ps_br", bufs=2, space="PSUM"))
        ps_y = ctx.enter_context(
            tc.tile_pool(name="ps_y", bufs=2, space="PSUM"))

        # ---- constants ----
        wcatT = const_pool.tile([128, NCH, NPROJ], dt.bfloat16)
        nc.sync.dma_start(wcatT[:], wcatT_ext[:])
        wmodT = const_pool.tile([128, 16, EMB], dt.float8e4)
        nc.scalar.dma_start(wmodT[:], wmodT_ext[:])
        cpk = const_pool.tile([128, 58], dt.float32)
        nc.sync.dma_start(cpk[:], cpack_ext[:].partition_broadcast(128))
        ident_bf = const_pool.tile([128, 128], dt.bfloat16)
        make_identity(nc, ident_bf[:])
        ident_f32 = const_pool.tile([128, 128], dt.float32)
        make_identity(nc, ident_f32[:])

        skew_c = cpk[:, 0:16]     # (conservA+bconv) - transpose, flattened
        diss_c = cpk[:, 16:32]    # dissA + bdiss, flattened
        eye16 = cpk[:, 32:48]     # flattened I4
        readin_c = cpk[:, 48:52]
        writeout_c = cpk[:, 52:56]

        xbf_dram = dram_pool.tile([B_loc, IN_DIM], dt.bfloat16)
        s_all = sm1_pool.tile([128, NT], dt.float32)
        proj_all = sm1_pool.tile([128, NT, NPROJ], dt.float32)
        E_all = sm1_pool.tile([128, NT, 16], dt.float32)
        c_all = sm1_pool.tile([128, NT, NS], dt.float32)
        ww_all = sm1_pool.tile([128, NT, NS], dt.float32)

        def bcast(ap2d, shape):
            return ap2d.unsqueeze(1).broadcast_to(shape)

        x_bfs = {}

        def p1_tile(t):
            """load + cast + sum-of-squares for tile t."""
            x_bf = xbb_pool.tile([128, IN_DIM], dt.bfloat16, tag="x_bf")
            x_bfs[t] = x_bf
            ss = small_pool.tile([128, 4], dt.float32, tag="ss")
            for q in range(4):
                xf = p1_pool.tile([128, EMB], dt.float32, tag="xf")
                nc.sync.dma_start(
                    xf[:], x_ext[t * 128:(t + 1) * 128,
                                 q * EMB:(q + 1) * EMB])
                if q % 2 == 0:
                    nc.vector.tensor_copy(x_bf[:, q * EMB:(q + 1) * EMB],
                                          xf[:])
                else:
                    nc.scalar.activation(x_bf[:, q * EMB:(q + 1) * EMB],
                                         xf[:], Act.Copy)
                sqj = str_pool.tile([128, EMB], dt.bfloat16, tag="tmp")
                nc.scalar.activation(sqj[:], xf[:], Act.Square,
                                     accum_out=ss[:, q:q + 1])
            nc.sync.dma_start(xbf_dram[t * 128:(t + 1) * 128, :], x_bf[:])
            s01 = small_pool.tile([128, 1], dt.float32, tag="s01")
            s23 = small_pool.tile([128, 1], dt.float32, tag="s23")
            nc.vector.tensor_add(s01[:], ss[:, 0:1], ss[:, 1:2])
            nc.vector.tensor_add(s23[:], ss[:, 2:3], ss[:, 3:4])
            nc.vector.tensor_add(s01[:], s01[:], s23[:])
            nc.vector.tensor_scalar(
                out=s01[:], in0=s01[:], scalar1=1.0 / IN_DIM,
                scalar2=EPS, op0=Alu.mult, op1=Alu.add)
            sqr = small_pool.tile([128, 1], dt.float32, tag="sqr")
            nc.scalar.activation(sqr[:], s01[:], Act.Sqrt)
            nc.vector.reciprocal(s_all[:, t:t + 1], sqr[:])

        def p3_proj(g):
            """proj.T for block g via strided fp32 loads + f32r matmul."""
            rows = slice(g * NB, (g + 1) * NB)
            proj_ps = ps_proj.tile([NPROJ, NB], dt.float32, tag="proj_ps")
            for c in range(NCH):
                xt = xt_pool.tile([128, NB], dt.bfloat16, tag="xt")
                nc.sync.dma_start(
                    xt[:], xbf_dram[rows, c * 128:(c + 1) * 128],
                    transpose=True)
                nc.tensor.matmul(proj_ps[:], wcatT[:, c, :], xt[:],
                                 start=(c == 0), stop=(c == NCH - 1))
            projT = sm1_pool.tile([NPROJ, NB], dt.float32, tag="projT")
            nc.vector.tensor_copy(projT[:], proj_ps[:])
            for i in range(TPB):
                t = g * TPB + i
                tr_ps = ps_trp.tile([128, NPROJ], dt.float32, tag="tr_ps")
                nc.tensor.transpose(
                    tr_ps[:], projT[:, i * 128:(i + 1) * 128],
                    ident_f32[:NPROJ, :NPROJ])
                nc.vector.tensor_scalar(
                    out=proj_all[:, t, :], in0=tr_ps[:],
                    scalar1=s_all[:, t:t + 1], scalar2=None, op0=Alu.mult)

        def p4_smalls(g):
            """per-row generator math for block g, batched over TPB tiles."""
            pb = proj_all[:, g * TPB:(g + 1) * TPB, :]   # [128,TPB,42]

            smw = small_pool.tile([128, TPB, 16], dt.float32, tag="smw")
            nc.vector.tensor_tensor(
                smw[:].rearrange("p t (i j) -> p t i j", j=NS),
                pb[:, :, 0:16].rearrange("p t (i j) -> p t i j", j=NS),
                pb[:, :, 0:16].rearrange("p t (j i) -> p t i j", i=NS),
                Alu.subtract)
            nc.vector.tensor_tensor(smw[:], smw[:],
                                    bcast(skew_c, [128, TPB, 16]), Alu.add)
            Rm = small_pool.tile([128, TPB, 16], dt.float32, tag="Rm")
            nc.vector.tensor_tensor(Rm[:], pb[:, :, 16:32],
                                    bcast(diss_c, [128, TPB, 16]), Alu.add)
            dtc = small_pool.tile([128, TPB, 1], dt.float32, tag="dtc")
            dtd = small_pool.tile([128, TPB, 1], dt.float32, tag="dtd")
            nc.scalar.activation(dtc[:], pb[:, :, 32:33], Act.Sigmoid,
                                 bias=cpk[:, 56:57])
            nc.scalar.activation(dtd[:], pb[:, :, 33:34], Act.Sigmoid,
                                 bias=cpk[:, 57:58])
            nc.vector.tensor_scalar(out=dtc[:], in0=dtc[:], scalar1=R_SIG,
                                    scalar2=C_SIG, op0=Alu.mult, op1=Alu.add)
            nc.vector.tensor_scalar(out=dtd[:], in0=dtd[:], scalar1=R_SIG,
                                    scalar2=C_SIG, op0=Alu.mult, op1=Alu.add)

            prod = small_pool.tile([128, TPB, 64], dt.float32, tag="prod")
            pv5 = prod[:].rearrange("p t (i j k) -> p t i j k", j=NS, k=NS)
            pvr = prod[:].rearrange("p t (ij k) -> p t ij k", k=NS)

            def mm_t(dst, lhs, rhs, rhs_pat):
                # batched per-row 4x4 matmul: loop j (broadcast dim) only
                lv = lhs[:].rearrange("p t (i k) -> p t i k", k=NS)
                rv = rhs[:].rearrange(rhs_pat, j=NS)
                for j in range(NS):
                    nc.vector.tensor_tensor(
                        pv5[:, :, :, j, :], lv,
                        rv[:, :, j, :].unsqueeze(2)
                        .broadcast_to([128, TPB, NS, NS]),
                        Alu.mult)
                nc.vector.tensor_reduce(dst[:], pvr, Axis.X, Alu.add)

            # K = R @ R^T
            Km = small_pool.tile([128, TPB, 16], dt.float32, tag="Km")
            mm_t(Km, Rm, Rm, "p t (j k) -> p t j k")
            # A = dtc*skew - dtd*K   (per-tile: dt scalars vary with t)
            Am = small_pool.tile([128, TPB, 16], dt.float32, tag="Am")
            for i in range(TPB):
                nc.vector.tensor_scalar(
                    out=Am[:, i, :], in0=Km[:, i, :],
                    scalar1=dtd[:, i, :], scalar2=None, op0=Alu.mult)
                nc.vector.scalar_tensor_tensor(
                    out=Am[:, i, :], in0=smw[:, i, :], scalar=dtc[:, i, :],
                    in1=Am[:, i, :], op0=Alu.mult, op1=Alu.subtract)
            # expm
            Em = small_pool.tile([128, TPB, 16], dt.float32, tag="Em")
            nc.vector.tensor_tensor(Em[:], Am[:],
                                    bcast(eye16, [128, TPB, 16]), Alu.add)
            term = small_pool.tile([128, TPB, 16], dt.float32, tag="term")
            term2 = small_pool.tile([128, TPB, 16], dt.float32, tag="term2")
            nc.vector.tensor_copy(term[:], Am[:])
            for k in range(2, 9):
                mm_t(term2, term, Am, "p t (k j) -> p t j k")
                nc.vector.tensor_scalar(out=term[:], in0=term2[:],
                                        scalar1=1.0 / k, scalar2=None,
                                        op0=Alu.mult)
                nc.vector.tensor_tensor(Em[:], Em[:], term[:], Alu.add)
            E2 = small_pool.tile([128, TPB, 16], dt.float32, tag="E2")
            cur, nxt = Em, E2
            for _ in range(4):
                mm_t(nxt, cur, cur, "p t (k j) -> p t j k")
                cur, nxt = nxt, cur
            nc.vector.tensor_copy(E_all[:, g * TPB:(g + 1) * TPB, :], cur[:])
            # rw / ww / c
            rw = small_pool.tile([128, TPB, NS], dt.float32, tag="rw")
            nc.vector.tensor_scalar(out=rw[:], in0=pb[:, :, 34:38],
                                    scalar1=scal["alpha_r"], scalar2=None,
                                    op0=Alu.mult)
            nc.vector.tensor_tensor(rw[:], rw[:],
                                    bcast(readin_c, [128, TPB, NS]), Alu.add)
            nc.scalar.activation(rw[:], rw[:], Act.Sigmoid)
            wws = ww_all[:, g * TPB:(g + 1) * TPB, :]
            nc.vector.tensor_scalar(out=wws, in0=pb[:, :, 38:42],
                                    scalar1=scal["alpha_w"], scalar2=None,
                                    op0=Alu.mult)
            nc.vector.tensor_tensor(wws, wws,
                                    bcast(writeout_c, [128, TPB, NS]),
                                    Alu.add)
            cprod = small_pool.tile([128, TPB, 16], dt.float32, tag="cprod")
            nc.vector.tensor_tensor(
                cprod[:].rearrange("p t (j n) -> p t j n", n=NS),
                cur[:].rearrange("p t (n j) -> p t j n", j=NS),
                rw[:].unsqueeze(2).broadcast_to([128, TPB, NS, NS]),
                Alu.mult)
            nc.vector.tensor_reduce(
                c_all[:, g * TPB:(g + 1) * TPB, :],
                cprod[:].rearrange("p t (j n) -> p t j n", n=NS),
                Axis.X, Alu.add)

        def p567_tile(t):
            x_bf = x_bfs.pop(t)
            # ---- P5: branch + PE transposes ----
            br = str_pool.tile([128, EMB], dt.bfloat16, tag="br")
            tmp = str_pool.tile([128, EMB], dt.bfloat16, tag="tmp")
            nc.vector.tensor_scalar(
                out=br[:], in0=x_bf[:, 3 * EMB:4 * EMB],
                scalar1=c_all[:, t, 3:4], scalar2=None, op0=Alu.mult)
            for j in (2, 1, 0):
                nc.vector.tensor_scalar(
                    out=tmp[:], in0=x_bf[:, j * EMB:(j + 1) * EMB],
                    scalar1=c_all[:, t, j:j + 1], scalar2=None, op0=Alu.mult)
                eng = nc.gpsimd if j == 1 else nc.vector
                eng.tensor_tensor(br[:], br[:], tmp[:], Alu.add)
            brT = brt_pool.tile([128, 16, 128], dt.float8e4, tag="brT")
            for h in range(16):
                br_ps = ps_br.tile([128, 128], dt.bfloat16, tag="br_ps")
                nc.tensor.transpose(br_ps[:], br[:, h * 128:(h + 1) * 128],
                                    ident_bf[:])
                if h % 2 == 0:
                    nc.scalar.activation(brT[:, h, :], br_ps[:], Act.Copy)
                else:
                    nc.vector.tensor_copy(brT[:, h, :], br_ps[:])
            # ---- P6: y = branch @ W_mod.T directly in [b, e] layout ----
            y_nb = str_pool.tile([128, EMB], dt.bfloat16, tag="y_nb")
            for eh in range(4):
                y_ps = ps_y.tile([128, 512], dt.float32, tag="y_ps")
                for c in range(16):
                    nc.tensor.matmul(
                        y_ps[:], brT[:, c, :],
                        wmodT[:, c, eh * 512:(eh + 1) * 512],
                        start=(c == 0), stop=(c == 15))
                if eh % 2 == 0:
                    nc.scalar.activation(y_nb[:, eh * 512:(eh + 1) * 512],
                                         y_ps[:], Act.Copy)
                else:
                    nc.vector.tensor_copy(y_nb[:, eh * 512:(eh + 1) * 512],
                                          y_ps[:])
            # ---- P7: out_n = sum_j E_nj x_j + ww_n y ----
            for n in range(NS):
                u = str_pool.tile([128, EMB], dt.bfloat16, tag="br")
                t2 = str_pool.tile([128, EMB], dt.bfloat16, tag="tmp")
                nc.vector.tensor_scalar(
                    out=u[:], in0=x_bf[:, 0:EMB],
                    scalar1=E_all[:, t, 4 * n:4 * n + 1], scalar2=None,
                    op0=Alu.mult)
                addeng = nc.gpsimd if n % 2 == 1 else nc.vector
                for j in (1, 2, 3):
                    nc.vector.tensor_scalar(
                        out=t2[:], in0=x_bf[:, j * EMB:(j + 1) * EMB],
                        scalar1=E_all[:, t, 4 * n + j:4 * n + j + 1],
                        scalar2=None, op0=Alu.mult)
                    addeng.tensor_tensor(u[:], u[:], t2[:], Alu.add)
                nc.scalar.activation(t2[:], y_nb[:], Act.Identity,
                                     scale=ww_all[:, t, n:n + 1])
                for hf in range(2):
                    sl = slice(hf * 1024, (hf + 1) * 1024)
                    ou = out_pool.tile([128, 1024], dt.float32, tag="ou")
                    nc.vector.tensor_tensor(ou[:], u[:, sl], t2[:, sl],
                                            Alu.add)
                    eng = nc.scalar if (2 * n + hf) % 2 == 0 else nc.sync
                    eng.dma_start(
                        out_ext[t * 128:(t + 1) * 128, n, sl], ou[:])

        # ---- schedule ----
        for i in range(TPB):
            p1_tile(i)
        for g in range(NBLK):
            p3_proj(g)
            p4_smalls(g)
            for i in range(TPB):
                if g + 1 < NBLK:
                    p1_tile((g + 1) * TPB + i)
                p567_tile(g * TPB + i)

    nc.compile()
    return nc


def _prep_weights(inputs):
    W_conv = np.asarray(inputs["W_conv"], np.float32)
    W_diss = np.asarray(inputs["W_diss"], np.float32)
    W_dtc = np.asarray(inputs["W_dtc"], np.float32)
    W_dtd = np.asarray(inputs["W_dtd"], np.float32)
    W_read = np.asarray(inputs["W_read"], np.float32)
    W_write = np.asarray(inputs["W_write"], np.float32)
    W_mod = np.asarray(inputs["W_mod"], np.float32)

    Wcat = np.concatenate([W_conv, W_diss, W_dtc, W_dtd, W_read, W_write],
                          axis=0)
    assert Wcat.shape == (NPROJ, IN_DIM)
    wcatT = np.ascontiguousarray(
        Wcat.T.reshape(IN_DIM // 128, 128, NPROJ).transpose(1, 0, 2)
    ).astype(BF16)
    # [k-within-chunk, c, e]: element [p,c,e] = W_mod.T[c*128+p, e]
    wmodT = np.ascontiguousarray(
        W_mod.T.reshape(16, 128, EMB).transpose(1, 0, 2)
    ).astype(ml_dtypes.float8_e4m3)

    scal = dict(
        bias_c=float(np.asarray(inputs["log_dt_c"]).reshape(-1)[0]
                     + np.asarray(inputs["b_dtc"]).reshape(-1)[0]),
        bias_d=float(np.asarray(inputs["log_dt_d"]).reshape(-1)[0]
                     + np.asarray(inputs["b_dtd"]).reshape(-1)[0]),
        alpha_r=float(np.asarray(inputs["alpha_read_in"]).reshape(-1)[0]),
        alpha_w=float(np.asarray(inputs["alpha_write_out"]).reshape(-1)[0]),
    )

    cM = np.asarray(inputs["conserv_A"], np.float32) + \
        np.asarray(inputs["b_conv"], np.float32).reshape(NS, NS)
    skew_const = (cM - cM.T).reshape(-1)
    dissC = (np.asarray(inputs["diss_A"], np.float32) +
             np.asarray(inputs["b_diss"], np.float32).reshape(NS, NS)
             ).reshape(-1)
    eye16 = np.eye(NS, dtype=np.float32).reshape(-1)
    readin = np.asarray(inputs["read_in"], np.float32).reshape(-1)
    writeout = np.asarray(inputs["write_out"], np.float32).reshape(-1)
    cpack = np.concatenate([
        skew_const, dissC, eye16, readin, writeout,
        np.array([scal["bias_c"], scal["bias_d"]], np.float32)]
    ).astype(np.float32)
    assert cpack.shape == (58,)
    return wcatT, wmodT, cpack, scal


_NC_CACHE = {}


def kernel(**inputs):
    from concourse.bass_utils import run_bass_kernel_spmd

    x = np.asarray(inputs["x"], np.float32)
    B = x.shape[0]
    B_loc = B // N_CORES
    wcatT, wmodT, cpack, scal = _prep_weights(inputs)

    key = (B_loc, tuple(sorted(scal.items())))
    if key not in _NC_CACHE:
        _NC_CACHE[key] = _build(B_loc, scal)
    nc = _NC_CACHE[key]

    xf = x.reshape(B, IN_DIM)
    in_maps = []
    for i in range(N_CORES):
        in_maps.append({
            "x": np.ascontiguousarray(xf[i * B_loc:(i + 1) * B_loc]),
            "wcatT": wcatT,
            "wmodT": wmodT,
            "cpack": cpack,
        })

    trace = os.environ.get("KERNEL_TRACE", "0") == "1"
    res = run_bass_kernel_spmd(nc, in_maps, core_ids=list(range(N_CORES)),
                               trace=trace)
    if trace and res.exec_time_ns is not None:
        print(f"HW exec time: {res.exec_time_ns} ns")
        kernel.last_exec_time_ns = res.exec_time_ns
    out = np.concatenate([res.results[i]["out"] for i in range(N_CORES)],
                         axis=0)
    return out



# revision 2
# speedup vs baseline: 1.6227x; 1.6227x over previous
"""Trainium2 Bass kernel for nn_ContinuousGenHyperConnections — v2.

Sharding: data-parallel over B=8192 across 8 NeuronCores (1024 rows each).
All weights replicated; no collectives.

v2 redesign vs baseline (908µs):
  - x loaded via gpsimd cast-DMA (fp32 HBM -> bf16 SBUF), no separate casts,
    no bf16 DRAM round-trip, no DMA transposes.
  - proj computed with PE-transposed x chunks as the STATIONARY operand and
    wcatT chunks moving (N=42), accumulating 64 chunks into a [128b, 42]
    PSUM tile -> proj lands directly in row-major layout, scaled by s on the
    ACT copy-out.
  - all stream mixing (branch, out) on DVE via fused scalar_tensor_tensor
    chains (no GpSimd adds).
  - y = branch @ W_mod.T with fp8 DoubleRow matmuls (brT fp8, wmodT fp8),
    halving PE column count.
  - out written bf16 -> fp32 via gpsimd cast-DMA.
"""

import os
import sys

sys.path.insert(0, "/opt/trn_rl_repo")

import numpy as np
import ml_dtypes

BF16 = ml_dtypes.bfloat16

DT_MIN, DT_MAX = 1e-3, 1.0
EPS = 1e-6
NS = 4  # streams
EMB = 2048
IN_DIM = 8192
N_CORES = 8
NPROJ = 42  # 16 conv + 16 diss + 1 dtc + 1 dtd + 4 read + 4 write
NCH = IN_DIM // 128  # 64 contraction chunks

CAST_DMA_IN = True    # gpsimd fp32->bf16 cast during load DMA
CAST_DMA_OUT = True   # gpsimd bf16->fp32 cast during store DMA
USE_DR = True         # fp8 DoubleRow for the inner-module matmul


def _build(B_loc, scal, num_devices=N_CORES):
    import concourse.bacc as bacc
    import concourse.mybir as mybir
    import concourse.tile as tile
    from concourse.masks import make_identity
    from contextlib import ExitStack

    dt = mybir.dt
    Alu = mybir.AluOpType
    Act = mybir.ActivationFunctionType
    Axis = mybir.AxisListType
    PM = mybir.MatmulPerfMode

    NT = B_loc // 128
    TPB = min(4, NT)          # tiles per smalls block
    NBLK = NT // TPB

    # expm 2^-4 prescale folded into dt: dt_eff = (DT_MIN + range*sig)/16
    R_SIG = (DT_MAX - DT_MIN) / 16.0
    C_SIG = DT_MIN / 16.0

    nc = bacc.Bacc("TRN2", target_bir_lowering=False, debug=False,
                   num_devices=num_devices)

    x_ext = nc.declare_dram_parameter("x", [B_loc, IN_DIM], dt.float32,
                                      isOutput=False)
    wcatT_ext = nc.declare_dram_parameter("wcatT", [128, NCH, NPROJ],
                                          dt.bfloat16, isOutput=False)
    wmodT_ext = nc.declare_dram_parameter("wmodT", [128, 16, EMB],
                                          dt.float8e4, isOutput=False)
    cpack_ext = nc.declare_dram_parameter("cpack", [58], dt.float32,
                                          isOutput=False)
    out_ext = nc.declare_dram_parameter("out", [B_loc, NS, EMB], dt.float32,
                                        isOutput=True)

    with tile.TileContext(nc) as tc, ExitStack() as ctx:
        const_pool = ctx.enter_context(tc.tile_pool(name="const", bufs=1))
        xbb_pool = ctx.enter_context(tc.tile_pool(name="xbb", bufs=5))
        xf_pool = ctx.enter_context(tc.tile_pool(name="xf", bufs=2))
        xt_pool = ctx.enter_context(tc.tile_pool(name="xt", bufs=2))
        small_pool = ctx.enter_context(tc.tile_pool(name="small", bufs=2))
        sm1_pool = ctx.enter_context(tc.tile_pool(name="sm1", bufs=1))
        str_pool = ctx.enter_context(tc.tile_pool(name="stream", bufs=2))
        brt_pool = ctx.enter_context(tc.tile_pool(name="brt", bufs=2))
        ou_pool = ctx.enter_context(tc.tile_pool(name="oup", bufs=4))
        ps_proj = ctx.enter_context(
            tc.tile_pool(name="ps_proj", bufs=1, space="PSUM"))
        ps_trp = ctx.enter_context(
            tc.tile_pool(name="ps_trp", bufs=2, space="PSUM"))
        ps_brt = ctx.enter_context(
            tc.tile_pool(name="ps_brt", bufs=1, space="PSUM"))
        ps_y = ctx.enter_context(
            tc.tile_pool(name="ps_y", bufs=1, space="PSUM"))

        # ---- constants ----
        wcatT = const_pool.tile([128, NCH, NPROJ], dt.bfloat16)
        nc.sync.dma_start(wcatT[:], wcatT_ext[:])
        wmodT = const_pool.tile([128, 16, EMB], dt.float8e4)
        nc.scalar.dma_start(wmodT[:], wmodT_ext[:])
        cpk = const_pool.tile([128, 58], dt.float32)
        nc.sync.dma_start(cpk[:], cpack_ext[:].partition_broadcast(128))
        ident_bf = const_pool.tile([128, 128], dt.bfloat16)
        make_identity(nc, ident_bf[:])

        skew_c = cpk[:, 0:16]     # (conservA+bconv) - transpose, flattened
        diss_c = cpk[:, 16:32]    # dissA + bdiss, flattened
        eye16 = cpk[:, 32:48]     # flattened I4
        readin_c = cpk[:, 48:52]
        writeout_c = cpk[:, 52:56]

        s_all = sm1_pool.tile([128, NT], dt.float32)
        proj_all = sm1_pool.tile([128, NT, NPROJ], dt.float32)
        E_all = sm1_pool.tile([128, NT, 16], dt.float32)
        c_all = sm1_pool.tile([128, NT, NS], dt.float32)
        ww_all = sm1_pool.tile([128, NT, NS], dt.float32)

        def bcast(ap2d, shape):
            return ap2d.unsqueeze(1).broadcast_to(shape)

        x_bfs = {}

        def p12_tile(t):
            """load+cast, sum-of-squares -> s, and proj for tile t."""
            rows = slice(t * 128, (t + 1) * 128)
            x_bf = xbb_pool.tile([128, IN_DIM], dt.bfloat16, tag="x_bf")
            x_bfs[t] = x_bf
            ss = small_pool.tile([128, 4], dt.float32, tag="ss")
            for q in range(4):
                xsl = slice(q * EMB, (q + 1) * EMB)
                if CAST_DMA_IN:
                    nc.gpsimd.dma_start(x_bf[:, xsl], x_ext[rows, xsl])
                else:
                    xf = xf_pool.tile([128, EMB], dt.float32, tag="xf")
                    eng = nc.sync if q % 2 == 0 else nc.scalar
                    eng.dma_start(xf[:], x_ext[rows, xsl])
                    if q % 2 == 0:
                        nc.vector.tensor_copy(x_bf[:, xsl], xf[:])
                    else:
                        nc.scalar.activation(x_bf[:, xsl], xf[:], Act.Copy)
                sqj = str_pool.tile([128, EMB], dt.bfloat16, tag="sqj")
                nc.scalar.activation(sqj[:], x_bf[:, xsl], Act.Square,
                                     accum_out=ss[:, q:q + 1])
            s01 = small_pool.tile([128, 1], dt.float32, tag="s01")
            s23 = small_pool.tile([128, 1], dt.float32, tag="s23")
            nc.vector.tensor_add(s01[:], ss[:, 0:1], ss[:, 1:2])
            nc.vector.tensor_add(s23[:], ss[:, 2:3], ss[:, 3:4])
            nc.vector.tensor_add(s01[:], s01[:], s23[:])
            nc.vector.tensor_scalar(
                out=s01[:], in0=s01[:], scalar1=1.0 / IN_DIM,
                scalar2=EPS, op0=Alu.mult, op1=Alu.add)
            sqr = small_pool.tile([128, 1], dt.float32, tag="sqr")
            nc.scalar.activation(sqr[:], s01[:], Act.Sqrt)
            nc.vector.reciprocal(s_all[:, t:t + 1], sqr[:])

            # ---- proj: xT chunks stationary, wcatT moving (N=42) ----
            proj_ps = ps_proj.tile([128, NPROJ], dt.float32, tag="proj_ps")
            for cg in range(16):
                tp_ps = ps_trp.tile([128, 512], dt.bfloat16, tag="tp_ps")
                for i in range(4):
                    c = cg * 4 + i
                    nc.tensor.transpose(
                        tp_ps[:, i * 128:(i + 1) * 128],
                        x_bf[:, c * 128:(c + 1) * 128], ident_bf[:])
                xt = xt_pool.tile([128, 512], dt.bfloat16, tag="xt")
                nc.scalar.activation(xt[:], tp_ps[:], Act.Copy)
                for i in range(4):
                    c = cg * 4 + i
                    nc.tensor.matmul(
                        proj_ps[:], xt[:, i * 128:(i + 1) * 128],
                        wcatT[:, c, :],
                        start=(c == 0), stop=(c == NCH - 1))
            # scaled copy-out: proj_all = s * proj
            nc.scalar.activation(proj_all[:, t, :], proj_ps[:], Act.Identity,
                                 scale=s_all[:, t:t + 1])

        def p4_smalls(g):
            """per-row generator math for block g, batched over TPB tiles."""
            pb = proj_all[:, g * TPB:(g + 1) * TPB, :]   # [128,TPB,42]

            smw = small_pool.tile([128, TPB, 16], dt.float32, tag="smw")
            nc.vector.tensor_tensor(
                smw[:].rearrange("p t (i j) -> p t i j", j=NS),
                pb[:, :, 0:16].rearrange("p t (i j) -> p t i j", j=NS),
                pb[:, :, 0:16].rearrange("p t (j i) -> p t i j", i=NS),
                Alu.subtract)
            nc.vector.tensor_tensor(smw[:], smw[:],
                                    bcast(skew_c, [128, TPB, 16]), Alu.add)
            Rm = small_pool.tile([128, TPB, 16], dt.float32, tag="Rm")
            nc.vector.tensor_tensor(Rm[:], pb[:, :, 16:32],
                                    bcast(diss_c, [128, TPB, 16]), Alu.add)
            dtc = small_pool.tile([128, TPB, 1], dt.float32, tag="dtc")
            dtd = small_pool.tile([128, TPB, 1], dt.float32, tag="dtd")
            nc.scalar.activation(dtc[:], pb[:, :, 32:33], Act.Sigmoid,
                                 bias=cpk[:, 56:57])
            nc.scalar.activation(dtd[:], pb[:, :, 33:34], Act.Sigmoid,
                                 bias=cpk[:, 57:58])
            nc.vector.tensor_scalar(out=dtc[:], in0=dtc[:], scalar1=R_SIG,
                                    scalar2=C_SIG, op0=Alu.mult, op1=Alu.add)
            nc.vector.tensor_scalar(out=dtd[:], in0=dtd[:], scalar1=R_SIG,
                                    scalar2=C_SIG, op0=Alu.mult, op1=Alu.add)

            prod = small_pool.tile([128, TPB, 64], dt.float32, tag="prod")
            pv5 = prod[:].rearrange("p t (i j k) -> p t i j k", j=NS, k=NS)
            pvr = prod[:].rearrange("p t (ij k) -> p t ij k", k=NS)

            def mm_t(dst, lhs, rhs, rhs_pat):
                # batched per-row 4x4 matmul: loop j (broadcast dim) only
                lv = lhs[:].rearrange("p t (i k) -> p t i k", k=NS)
                rv = rhs[:].rearrange(rhs_pat, j=NS)
                for j in range(NS):
                    nc.vector.tensor_tensor(
                        pv5[:, :, :, j, :], lv,
                        rv[:, :, j, :].unsqueeze(2)
                        .broadcast_to([128, TPB, NS, NS]),
                        Alu.mult)
                nc.vector.tensor_reduce(dst[:], pvr, Axis.X, Alu.add)

            # K = R @ R^T
            Km = small_pool.tile([128, TPB, 16], dt.float32, tag="Km")
            mm_t(Km, Rm, Rm, "p t (j k) -> p t j k")
            # A = dtc*skew - dtd*K   (per-tile: dt scalars vary with t)
            Am = small_pool.tile([128, TPB, 16], dt.float32, tag="Am")
            for i in range(TPB):
                nc.vector.tensor_scalar(
                    out=Am[:, i, :], in0=Km[:, i, :],
                    scalar1=dtd[:, i, :], scalar2=None, op0=Alu.mult)
                nc.vector.scalar_tensor_tensor(
                    out=Am[:, i, :], in0=smw[:, i, :], scalar=dtc[:, i, :],
                    in1=Am[:, i, :], op0=Alu.mult, op1=Alu.subtract)
            # expm
            Em = small_pool.tile([128, TPB, 16], dt.float32, tag="Em")
            nc.vector.tensor_tensor(Em[:], Am[:],
                                    bcast(eye16, [128, TPB, 16]), Alu.add)
            term = small_pool.tile([128, TPB, 16], dt.float32, tag="term")
            term2 = small_pool.tile([128, TPB, 16], dt.float32, tag="term2")
            nc.vector.tensor_copy(term[:], Am[:])
            for k in range(2, 9):
                mm_t(term2, term, Am, "p t (k j) -> p t j k")
                nc.vector.tensor_scalar(out=term[:], in0=term2[:],
                                        scalar1=1.0 / k, scalar2=None,
                                        op0=Alu.mult)
                nc.vector.tensor_tensor(Em[:], Em[:], term[:], Alu.add)
            E2 = small_pool.tile([128, TPB, 16], dt.float32, tag="E2")
            cur, nxt = Em, E2
            for _ in range(4):
                mm_t(nxt, cur, cur, "p t (k j) -> p t j k")
                cur, nxt = nxt, cur
            nc.vector.tensor_copy(E_all[:, g * TPB:(g + 1) * TPB, :], cur[:])
            # rw / ww / c
            rw = small_pool.tile([128, TPB, NS], dt.float32, tag="rw")
            nc.vector.tensor_scalar(out=rw[:], in0=pb[:, :, 34:38],
                                    scalar1=scal["alpha_r"], scalar2=None,
                                    op0=Alu.mult)
            nc.vector.tensor_tensor(rw[:], rw[:],
                                    bcast(readin_c, [128, TPB, NS]), Alu.add)
            nc.scalar.activation(rw[:], rw[:], Act.Sigmoid)
            wws = ww_all[:, g * TPB:(g + 1) * TPB, :]
            nc.vector.tensor_scalar(out=wws, in0=pb[:, :, 38:42],
                                    scalar1=scal["alpha_w"], scalar2=None,
                                    op0=Alu.mult)
            nc.vector.tensor_tensor(wws, wws,
                                    bcast(writeout_c, [128, TPB, NS]),
                                    Alu.add)
            cprod = small_pool.tile([128, TPB, 16], dt.float32, tag="cprod")
            nc.vector.tensor_tensor(
                cprod[:].rearrange("p t (j n) -> p t j n", n=NS),
                cur[:].rearrange("p t (n j) -> p t j n", j=NS),
                rw[:].unsqueeze(2).broadcast_to([128, TPB, NS, NS]),
                Alu.mult)
            nc.vector.tensor_reduce(
                c_all[:, g * TPB:(g + 1) * TPB, :],
                cprod[:].rearrange("p t (j n) -> p t j n", n=NS),
                Axis.X, Alu.add)

        def p567_tile(t):
            x_bf = x_bfs.pop(t)
            # ---- P5: branch = sum_j c_j x_j (ACT start + DVE stt chain) ----
            br = str_pool.tile([128, EMB], dt.bfloat16, tag="br")
            nc.scalar.activation(br[:], x_bf[:, 0:EMB], Act.Identity,
                                 scale=c_all[:, t, 0:1])
            for j in (1, 2, 3):
                nc.vector.scalar_tensor_tensor(
                    out=br[:], in0=x_bf[:, j * EMB:(j + 1) * EMB],
                    scalar=c_all[:, t, j:j + 1], in1=br[:],
                    op0=Alu.mult, op1=Alu.add)
            # branchT chunks: 16 PE transposes -> 2 psum groups -> fp8 sbuf
            brT = brt_pool.tile([128, 16, 128], dt.float8e4, tag="brT")
            for hg in range(2):
                bt_ps = ps_brt.tile([128, 1024], dt.bfloat16, tag="bt_ps")
                for i in range(8):
                    h = hg * 8 + i
                    nc.tensor.transpose(
                        bt_ps[:, i * 128:(i + 1) * 128],
                        br[:, h * 128:(h + 1) * 128], ident_bf[:])
                nc.scalar.activation(
                    brT[:, hg * 8:(hg + 1) * 8, :], bt_ps[:], Act.Copy)
            # ---- P6: y = branch @ W_mod.T (fp8 DoubleRow) ----
            y_nb = str_pool.tile([128, EMB], dt.bfloat16, tag="y_nb")
            if USE_DR:
                y_ps = []
                for eh in range(4):
                    yp = ps_y.tile([128, 512], dt.float32, tag=f"y{eh}")
                    y_ps.append(yp)
                for kt in range(8):
                    lhsT = brT[:, 2 * kt:2 * kt + 2, :]
                    for eh in range(4):
                        nc.tensor.matmul(
                            y_ps[eh][:], lhsT,
                            wmodT[:, 2 * kt:2 * kt + 2,
                                  eh * 512:(eh + 1) * 512],
                            start=(kt == 0), stop=(kt == 7),
                            perf_mode=PM.DoubleRow)
                for eh in range(4):
                    nc.scalar.activation(y_nb[:, eh * 512:(eh + 1) * 512],
                                         y_ps[eh][:], Act.Copy)
            else:
                for eh in range(4):
                    y_ps = ps_y.tile([128, 512], dt.float32, tag="y_ps")
                    for c in range(16):
                        nc.tensor.matmul(
                            y_ps[:], brT[:, c, :],
                            wmodT[:, c, eh * 512:(eh + 1) * 512],
                            start=(c == 0), stop=(c == 15))
                    nc.scalar.activation(y_nb[:, eh * 512:(eh + 1) * 512],
                                         y_ps[:], Act.Copy)
            # ---- P7: out_n = sum_j E_nj x_j + ww_n y ----
            for n in range(NS):
                ou = ou_pool.tile([128, EMB], dt.bfloat16, tag="ou")
                nc.scalar.activation(ou[:], x_bf[:, 0:EMB], Act.Identity,
                                     scale=E_all[:, t, 4 * n:4 * n + 1])
                for j in (1, 2, 3):
                    nc.vector.scalar_tensor_tensor(
                        out=ou[:], in0=x_bf[:, j * EMB:(j + 1) * EMB],
                        scalar=E_all[:, t, 4 * n + j:4 * n + j + 1],
                        in1=ou[:], op0=Alu.mult, op1=Alu.add)
                nc.vector.scalar_tensor_tensor(
                    out=ou[:], in0=y_nb[:],
                    scalar=ww_all[:, t, n:n + 1], in1=ou[:],
                    op0=Alu.mult, op1=Alu.add)
                if CAST_DMA_OUT:
                    nc.gpsimd.dma_start(
                        out_ext[t * 128:(t + 1) * 128, n, :], ou[:])
                else:
                    of = ou_pool.tile([128, EMB], dt.float32, tag="of")
                    nc.vector.tensor_copy(of[:], ou[:])
                    eng = nc.scalar if n % 2 == 0 else nc.sync
                    eng.dma_start(
                        out_ext[t * 128:(t + 1) * 128, n, :], of[:])

        # ---- schedule ----
        for i in range(TPB):
            p12_tile(i)
        for g in range(NBLK):
            p4_smalls(g)
            for i in range(TPB):
                if g + 1 < NBLK:
                    p12_tile((g + 1) * TPB + i)
                p567_tile(g * TPB + i)

    nc.compile()
    return nc


def _prep_weights(inputs):
    W_conv = np.asarray(inputs["W_conv"], np.float32)
    W_diss = np.asarray(inputs["W_diss"], np.float32)
    W_dtc = np.asarray(inputs["W_dtc"], np.float32)
    W_dtd = np.asarray(inputs["W_dtd"], np.float32)
    W_read = np.asarray(inputs["W_read"], np.float32)
    W_write = np.asarray(inputs["W_write"], np.float32)
    W_mod = np.asarray(inputs["W_mod"], np.float32)

    Wcat = np.concatenate([W_conv, W_diss, W_dtc, W_dtd, W_read, W_write],
                          axis=0)
    assert Wcat.shape == (NPROJ, IN_DIM)
    wcatT = np.ascontiguousarray(
        Wcat.T.reshape(IN_DIM // 128, 128, NPROJ).transpose(1, 0, 2)
    ).astype(BF16)
    # [k-within-chunk, c, e]: element [p,c,e] = W_mod.T[c*128+p, e]
    wmodT = np.ascontiguousarray(
        W_mod.T.reshape(16, 128, EMB).transpose(1, 0, 2)
    ).astype(ml_dtypes.float8_e4m3)

    scal = dict(
        bias_c=float(np.asarray(inputs["log_dt_c"]).reshape(-1)[0]
                     + np.asarray(inputs["b_dtc"]).reshape(-1)[0]),
        bias_d=float(np.asarray(inputs["log_dt_d"]).reshape(-1)[0]
                     + np.asarray(inputs["b_dtd"]).reshape(-1)[0]),
        alpha_r=float(np.asarray(inputs["alpha_read_in"]).reshape(-1)[0]),
        alpha_w=float(np.asarray(inputs["alpha_write_out"]).reshape(-1)[0]),
    )

    cM = np.asarray(inputs["conserv_A"], np.float32) + \
        np.asarray(inputs["b_conv"], np.float32).reshape(NS, NS)
    skew_const = (cM - cM.T).reshape(-1)
    dissC = (np.asarray(inputs["diss_A"], np.float32) +
             np.asarray(inputs["b_diss"], np.float32).reshape(NS, NS)
             ).reshape(-1)
    eye16 = np.eye(NS, dtype=np.float32).reshape(-1)
    readin = np.asarray(inputs["read_in"], np.float32).reshape(-1)
    writeout = np.asarray(inputs["write_out"], np.float32).reshape(-1)
    cpack = np.concatenate([
        skew_const, dissC, eye16, readin, writeout,
        np.array([scal["bias_c"], scal["bias_d"]], np.float32)]
    ).astype(np.float32)
    assert cpack.shape == (58,)
    return wcatT, wmodT, cpack, scal


_NC_CACHE = {}


def kernel(**inputs):
    from concourse.bass_utils import run_bass_kernel_spmd

    x = np.asarray(inputs["x"], np.float32)
    B = x.shape[0]
    B_loc = B // N_CORES
    wcatT, wmodT, cpack, scal = _prep_weights(inputs)

    key = (B_loc, tuple(sorted(scal.items())))
    if key not in _NC_CACHE:
        _NC_CACHE[key] = _build(B_loc, scal)
    nc = _NC_CACHE[key]

    xf = x.reshape(B, IN_DIM)
    in_maps = []
    for i in range(N_CORES):
        in_maps.append({
            "x": np.ascontiguousarray(xf[i * B_loc:(i + 1) * B_loc]),
            "wcatT": wcatT,
            "wmodT": wmodT,
            "cpack": cpack,
        })

    trace = os.environ.get("KERNEL_TRACE", "0") == "1"
    res = run_bass_kernel_spmd(nc, in_maps, core_ids=list(range(N_CORES)),
                               trace=trace)
    if trace and res.exec_time_ns is not None:
        print(f"HW exec time: {res.exec_time_ns} ns")
        kernel.last_exec_time_ns = res.exec_time_ns
    out = np.concatenate([res.results[i]["out"] for i in range(N_CORES)],
                         axis=0)
    return out


# revision 5
# speedup vs baseline: 1.9066x; 1.1749x over previous
"""Trainium2 Bass kernel for nn_ContinuousGenHyperConnections — v3.

Sharding: data-parallel over B=8192 across 8 NeuronCores (1024 rows each).

v3 vs v2 (560us):
  - scalar_tensor_tensor runs 1x on DVE (2238ns) -> replaced by
    tensor_scalar (4x, 594ns) + tensor_tensor (2x, 1127ns) pairs.
  - out-streams 2,3 mixed on the PE via diagonal-stationary matmuls
    accumulating in PSUM (per-row scalars via diag matrices).
  - proj per 4-tile block: wcatT stationary (42 rows), xT moving N=512.
  - sum-of-squares off ACT (1x on bf16): 2 chunks gpsimd stt-accum,
    2 chunks DVE tensor_tensor_reduce.
  - ACT only does psum->sbuf copies, scaled copies, sigmoids.
"""

import os
import sys

sys.path.insert(0, "/opt/trn_rl_repo")

import numpy as np
import ml_dtypes

BF16 = ml_dtypes.bfloat16

DT_MIN, DT_MAX = 1e-3, 1.0
EPS = 1e-6
NS = 4
EMB = 2048
IN_DIM = 8192
N_CORES = 8
NPROJ = 42
NCH = IN_DIM // 128

CAST_DMA_IN = True
CAST_DMA_OUT = True
PE_STREAMS = (2, 3)   # out-streams mixed on the PE (rest on DVE)


def _build(B_loc, scal, num_devices=N_CORES):
    import concourse.bacc as bacc
    import concourse.mybir as mybir
    import concourse.tile as tile
    from concourse.masks import make_identity
    from contextlib import ExitStack

    dt = mybir.dt
    Alu = mybir.AluOpType
    Act = mybir.ActivationFunctionType
    Axis = mybir.AxisListType
    PM = mybir.MatmulPerfMode

    NT = B_loc // 128
    TPB = min(4, NT)
    NBLK = NT // TPB

    R_SIG = (DT_MAX - DT_MIN) / 16.0
    C_SIG = DT_MIN / 16.0

    nc = bacc.Bacc("TRN2", target_bir_lowering=False, debug=False,
                   num_devices=num_devices)

    x_ext = nc.declare_dram_parameter("x", [B_loc, IN_DIM], dt.float32,
                                      isOutput=False)
    wcatT_ext = nc.declare_dram_parameter("wcatT", [128, NCH, NPROJ],
                                          dt.bfloat16, isOutput=False)
    wmodT_ext = nc.declare_dram_parameter("wmodT", [128, 16, EMB],
                                          dt.float8e4, isOutput=False)
    cpack_ext = nc.declare_dram_parameter("cpack", [58], dt.float32,
                                          isOutput=False)
    out_ext = nc.declare_dram_parameter("out", [B_loc, NS, EMB], dt.float32,
                                        isOutput=True)

    with tile.TileContext(nc) as tc, ExitStack() as ctx:
        const_pool = ctx.enter_context(tc.tile_pool(name="const", bufs=1))
        xbb_pool = ctx.enter_context(tc.tile_pool(name="xbb", bufs=6))
        xf_pool = ctx.enter_context(tc.tile_pool(name="xf", bufs=2))
        xt_pool = ctx.enter_context(tc.tile_pool(name="xt", bufs=2))
        small_pool = ctx.enter_context(tc.tile_pool(name="small", bufs=2))
        sm1_pool = ctx.enter_context(tc.tile_pool(name="sm1", bufs=1))
        str_pool = ctx.enter_context(tc.tile_pool(name="stream", bufs=2))
        brt_pool = ctx.enter_context(tc.tile_pool(name="brt", bufs=2))
        ou_pool = ctx.enter_context(tc.tile_pool(name="oup", bufs=4))
        ps_proj = ctx.enter_context(
            tc.tile_pool(name="ps_proj", bufs=1, space="PSUM"))
        ps_tr = ctx.enter_context(
            tc.tile_pool(name="ps_tr", bufs=1, space="PSUM"))
        ps_trp = ctx.enter_context(
            tc.tile_pool(name="ps_trp", bufs=2, space="PSUM"))
        ps_brt = ctx.enter_context(
            tc.tile_pool(name="ps_brt", bufs=1, space="PSUM"))
        ps_y = ctx.enter_context(
            tc.tile_pool(name="ps_y", bufs=1, space="PSUM"))

        # ---- constants ----
        wcatT = const_pool.tile([128, NCH, NPROJ], dt.bfloat16)
        nc.sync.dma_start(wcatT[:], wcatT_ext[:])
        wmodT = const_pool.tile([128, 16, EMB], dt.float8e4)
        nc.scalar.dma_start(wmodT[:], wmodT_ext[:])
        cpk = const_pool.tile([128, 58], dt.float32)
        nc.sync.dma_start(cpk[:], cpack_ext[:].partition_broadcast(128))
        ident_bf = const_pool.tile([128, 128], dt.bfloat16)
        make_identity(nc, ident_bf[:])
        ident_f32 = const_pool.tile([64, 64], dt.float32)
        make_identity(nc, ident_f32[:])

        skew_c = cpk[:, 0:16]
        diss_c = cpk[:, 16:32]
        eye16 = cpk[:, 32:48]
        readin_c = cpk[:, 48:52]
        writeout_c = cpk[:, 52:56]

        s_all = sm1_pool.tile([128, NT], dt.float32)
        proj_all = sm1_pool.tile([128, NT, NPROJ], dt.float32)
        E_all = sm1_pool.tile([128, NT, 16], dt.float32)
        c_all = sm1_pool.tile([128, NT, NS], dt.float32)
        ww_all = sm1_pool.tile([128, NT, NS], dt.float32)

        def bcast(ap2d, shape):
            return ap2d.unsqueeze(1).broadcast_to(shape)

        x_bfs = {}

        def p1_tile(t):
            """load (+cast) and rms stats for tile t."""
            rows = slice(t * 128, (t + 1) * 128)
            x_bf = xbb_pool.tile([128, IN_DIM], dt.bfloat16, tag="x_bf")
            x_bfs[t] = x_bf
            ss = small_pool.tile([128, 4], dt.float32, tag="ss")
            for q in range(4):
                xsl = slice(q * EMB, (q + 1) * EMB)
                if CAST_DMA_IN:
                    nc.gpsimd.dma_start(x_bf[:, xsl], x_ext[rows, xsl])
                else:
                    xf = xf_pool.tile([128, EMB], dt.float32, tag="xf")
                    eng = nc.sync if q % 2 == 0 else nc.scalar
                    eng.dma_start(xf[:], x_ext[rows, xsl])
                    if q % 2 == 0:
                        nc.vector.tensor_copy(x_bf[:, xsl], xf[:])
                    else:
                        nc.scalar.activation(x_bf[:, xsl], xf[:], Act.Copy)
                sqj = str_pool.tile([128, EMB], dt.bfloat16, tag="sqj")
                nc.scalar.activation(sqj[:], x_bf[:, xsl], Act.Square,
                                     accum_out=ss[:, q:q + 1])
            s01 = small_pool.tile([128, 1], dt.float32, tag="s01")
            s23 = small_pool.tile([128, 1], dt.float32, tag="s23")
            nc.vector.tensor_add(s01[:], ss[:, 0:1], ss[:, 1:2])
            nc.vector.tensor_add(s23[:], ss[:, 2:3], ss[:, 3:4])
            nc.vector.tensor_add(s01[:], s01[:], s23[:])
            nc.vector.tensor_scalar(
                out=s01[:], in0=s01[:], scalar1=1.0 / IN_DIM,
                scalar2=EPS, op0=Alu.mult, op1=Alu.add)
            sqr = small_pool.tile([128, 1], dt.float32, tag="sqr")
            nc.scalar.activation(sqr[:], s01[:], Act.Sqrt)
            nc.vector.reciprocal(s_all[:, t:t + 1], sqr[:])

        def proj_block(g):
            """projT [42, 512] over a 4-tile block; wcatT stationary."""
            NB = TPB * 128
            projT_ps = ps_proj.tile([NPROJ, NB], dt.float32, tag="projT_ps")
            for c in range(NCH):
                tp_ps = ps_trp.tile([128, NB], dt.bfloat16, tag="tp_ps")
                for i in range(TPB):
                    nc.tensor.transpose(
                        tp_ps[:, i * 128:(i + 1) * 128],
                        x_bfs[g * TPB + i][:, c * 128:(c + 1) * 128],
                        ident_bf[:])
                xt = xt_pool.tile([128, NB], dt.bfloat16, tag="xt")
                nc.scalar.activation(xt[:], tp_ps[:], Act.Copy)
                nc.tensor.matmul(projT_ps[:], wcatT[:, c, :], xt[:],
                                 start=(c == 0), stop=(c == NCH - 1))
            projT_sb = sm1_pool.tile([NPROJ, NB], dt.float32, tag="projT_sb")
            nc.scalar.activation(projT_sb[:], projT_ps[:], Act.Copy)
            for i in range(TPB):
                t = g * TPB + i
                tr_ps = ps_tr.tile([128, NPROJ], dt.float32, tag="tr_ps")
                nc.tensor.transpose(
                    tr_ps[:], projT_sb[:, i * 128:(i + 1) * 128],
                    ident_f32[:NPROJ, :NPROJ])
                nc.scalar.activation(proj_all[:, t, :], tr_ps[:],
                                     Act.Identity, scale=s_all[:, t:t + 1])

        def p4_smalls(g):
            """per-row generator math for block g, batched over TPB tiles."""
            pb = proj_all[:, g * TPB:(g + 1) * TPB, :]

            smw = small_pool.tile([128, TPB, 16], dt.float32, tag="smw")
            nc.vector.tensor_tensor(
                smw[:].rearrange("p t (i j) -> p t i j", j=NS),
                pb[:, :, 0:16].rearrange("p t (i j) -> p t i j", j=NS),
                pb[:, :, 0:16].rearrange("p t (j i) -> p t i j", i=NS),
                Alu.subtract)
            nc.vector.tensor_tensor(smw[:], smw[:],
                                    bcast(skew_c, [128, TPB, 16]), Alu.add)
            Rm = small_pool.tile([128, TPB, 16], dt.float32, tag="Rm")
            nc.vector.tensor_tensor(Rm[:], pb[:, :, 16:32],
                                    bcast(diss_c, [128, TPB, 16]), Alu.add)
            dtc = small_pool.tile([128, TPB, 1], dt.float32, tag="dtc")
            dtd = small_pool.tile([128, TPB, 1], dt.float32, tag="dtd")
            nc.scalar.activation(dtc[:], pb[:, :, 32:33], Act.Sigmoid,
                                 bias=cpk[:, 56:57])
            nc.scalar.activation(dtd[:], pb[:, :, 33:34], Act.Sigmoid,
                                 bias=cpk[:, 57:58])
            nc.vector.tensor_scalar(out=dtc[:], in0=dtc[:], scalar1=R_SIG,
                                    scalar2=C_SIG, op0=Alu.mult, op1=Alu.add)
            nc.vector.tensor_scalar(out=dtd[:], in0=dtd[:], scalar1=R_SIG,
                                    scalar2=C_SIG, op0=Alu.mult, op1=Alu.add)

            prod = small_pool.tile([128, TPB, 64], dt.float32, tag="prod")
            pv5 = prod[:].rearrange("p t (i j k) -> p t i j k", j=NS, k=NS)
            pvr = prod[:].rearrange("p t (ij k) -> p t ij k", k=NS)

            def mm_t(dst, lhs, rhs, rhs_pat):
                lv = lhs[:].rearrange("p t (i k) -> p t i k", k=NS)
                rv = rhs[:].rearrange(rhs_pat, j=NS)
                for j in range(NS):
                    nc.vector.tensor_tensor(
                        pv5[:, :, :, j, :], lv,
                        rv[:, :, j, :].unsqueeze(2)
                        .broadcast_to([128, TPB, NS, NS]),
                        Alu.mult)
                nc.vector.tensor_reduce(dst[:], pvr, Axis.X, Alu.add)

            Km = small_pool.tile([128, TPB, 16], dt.float32, tag="Km")
            mm_t(Km, Rm, Rm, "p t (j k) -> p t j k")
            Am = small_pool.tile([128, TPB, 16], dt.float32, tag="Am")
            for i in range(TPB):
                nc.vector.tensor_scalar(
                    out=Am[:, i, :], in0=Km[:, i, :],
                    scalar1=dtd[:, i, :], scalar2=None, op0=Alu.mult)
                nc.vector.scalar_tensor_tensor(
                    out=Am[:, i, :], in0=smw[:, i, :], scalar=dtc[:, i, :],
                    in1=Am[:, i, :], op0=Alu.mult, op1=Alu.subtract)
            Em = small_pool.tile([128, TPB, 16], dt.float32, tag="Em")
            nc.vector.tensor_tensor(Em[:], Am[:],
                                    bcast(eye16, [128, TPB, 16]), Alu.add)
            term = small_pool.tile([128, TPB, 16], dt.float32, tag="term")
            term2 = small_pool.tile([128, TPB, 16], dt.float32, tag="term2")
            nc.vector.tensor_copy(term[:], Am[:])
            for k in range(2, 9):
                mm_t(term2, term, Am, "p t (k j) -> p t j k")
                nc.vector.tensor_scalar(out=term[:], in0=term2[:],
                                        scalar1=1.0 / k, scalar2=None,
                                        op0=Alu.mult)
                nc.vector.tensor_tensor(Em[:], Em[:], term[:], Alu.add)
            E2 = small_pool.tile([128, TPB, 16], dt.float32, tag="E2")
            cur, nxt = Em, E2
            for _ in range(4):
                mm_t(nxt, cur, cur, "p t (k j) -> p t j k")
                cur, nxt = nxt, cur
            nc.vector.tensor_copy(E_all[:, g * TPB:(g + 1) * TPB, :], cur[:])
            rw = small_pool.tile([128, TPB, NS], dt.float32, tag="rw")
            nc.vector.tensor_scalar(out=rw[:], in0=pb[:, :, 34:38],
                                    scalar1=scal["alpha_r"], scalar2=None,
                                    op0=Alu.mult)
            nc.vector.tensor_tensor(rw[:], rw[:],
                                    bcast(readin_c, [128, TPB, NS]), Alu.add)
            nc.scalar.activation(rw[:], rw[:], Act.Sigmoid)
            wws = ww_all[:, g * TPB:(g + 1) * TPB, :]
            nc.vector.tensor_scalar(out=wws, in0=pb[:, :, 38:42],
                                    scalar1=scal["alpha_w"], scalar2=None,
                                    op0=Alu.mult)
            nc.vector.tensor_tensor(wws, wws,
                                    bcast(writeout_c, [128, TPB, NS]),
                                    Alu.add)
            cprod = small_pool.tile([128, TPB, 16], dt.float32, tag="cprod")
            nc.vector.tensor_tensor(
                cprod[:].rearrange("p t (j n) -> p t j n", n=NS),
                cur[:].rearrange("p t (n j) -> p t j n", j=NS),
                rw[:].unsqueeze(2).broadcast_to([128, TPB, NS, NS]),
                Alu.mult)
            nc.vector.tensor_reduce(
                c_all[:, g * TPB:(g + 1) * TPB, :],
                cprod[:].rearrange("p t (j n) -> p t j n", n=NS),
                Axis.X, Alu.add)

        def p567_tile(t):
            x_bf = x_bfs.pop(t)
            # ---- P5: branch = sum_j c_j x_j on DVE (TS + TT) ----
            br = str_pool.tile([128, EMB], dt.bfloat16, tag="br")
            tmp = str_pool.tile([128, EMB], dt.bfloat16, tag="tmp")
            nc.vector.tensor_scalar(out=br[:], in0=x_bf[:, 0:EMB],
                                    scalar1=c_all[:, t, 0:1], scalar2=None,
                                    op0=Alu.mult)
            for j in (1, 2, 3):
                nc.vector.tensor_scalar(
                    out=tmp[:], in0=x_bf[:, j * EMB:(j + 1) * EMB],
                    scalar1=c_all[:, t, j:j + 1], scalar2=None, op0=Alu.mult)
                nc.vector.tensor_tensor(br[:], br[:], tmp[:], Alu.add)
            # branchT: 16 PE transposes -> fp8 sbuf (2 groups of 8)
            brT = brt_pool.tile([128, 16, 128], dt.float8e4, tag="brT")
            for hg in range(2):
                bt_ps = ps_brt.tile([128, 1024], dt.bfloat16, tag="bt_ps")
                for i in range(8):
                    h = hg * 8 + i
                    nc.tensor.transpose(
                        bt_ps[:, i * 128:(i + 1) * 128],
                        br[:, h * 128:(h + 1) * 128], ident_bf[:])
                nc.scalar.activation(
                    brT[:, hg * 8:(hg + 1) * 8, :], bt_ps[:], Act.Copy)
            # ---- P6: y = branch @ W_mod.T (fp8 DoubleRow, 2 banks) ----
            y_nb = str_pool.tile([128, EMB], dt.bfloat16, tag="y_nb")
            for half in range(2):
                yps = []
                for i in range(2):
                    yp = ps_y.tile([128, 512], dt.float32, tag=f"y{i}")
                    yps.append(yp)
                for kt in range(8):
                    lhsT = brT[:, 2 * kt:2 * kt + 2, :]
                    for i in range(2):
                        eh = 2 * half + i
                        nc.tensor.matmul(
                            yps[i][:], lhsT,
                            wmodT[:, 2 * kt:2 * kt + 2,
                                  eh * 512:(eh + 1) * 512],
                            start=(kt == 0), stop=(kt == 7),
                            perf_mode=PM.DoubleRow)
                for i in range(2):
                    eh = 2 * half + i
                    nc.scalar.activation(y_nb[:, eh * 512:(eh + 1) * 512],
                                         yps[i][:], Act.Copy)
            # ---- diag matrices for PE-mixed streams ----
            diag = brt_pool.tile([128, 10, 128], dt.bfloat16, tag="diag")
            for di, n in enumerate(PE_STREAMS):
                for j in range(NS):
                    nc.vector.tensor_scalar(
                        out=diag[:, 5 * di + j, :], in0=ident_bf[:],
                        scalar1=E_all[:, t, 4 * n + j:4 * n + j + 1],
                        scalar2=None, op0=Alu.mult)
                nc.vector.tensor_scalar(
                    out=diag[:, 5 * di + 4, :], in0=ident_bf[:],
                    scalar1=ww_all[:, t, n:n + 1], scalar2=None, op0=Alu.mult)
            # ---- P7 ----
            for di, n in enumerate(PE_STREAMS):
                ou = ou_pool.tile([128, EMB], dt.bfloat16, tag="ou")
                for q in range(4):
                    qsl = slice(q * 512, (q + 1) * 512)
                    mx = ps_y.tile([128, 512], dt.float32, tag=f"y{q % 2}")
                    for term in range(5):
                        src = (y_nb[:, qsl] if term == 4 else
                               x_bf[:, term * EMB + q * 512:
                                    term * EMB + (q + 1) * 512])
                        nc.tensor.matmul(
                            mx[:], diag[:, 5 * di + term, :], src,
                            start=(term == 0), stop=(term == 4))
                    nc.scalar.activation(ou[:, qsl], mx[:], Act.Copy)
                if CAST_DMA_OUT:
                    nc.gpsimd.dma_start(
                        out_ext[t * 128:(t + 1) * 128, n, :], ou[:])
                else:
                    of = ou_pool.tile([128, EMB], dt.float32, tag="of")
                    nc.vector.tensor_copy(of[:], ou[:])
                    eng = nc.scalar if n % 2 == 0 else nc.sync
                    eng.dma_start(
                        out_ext[t * 128:(t + 1) * 128, n, :], of[:])
            for n in range(NS):
                if n in PE_STREAMS:
                    continue
                ou = ou_pool.tile([128, EMB], dt.bfloat16, tag="ou")
                nc.vector.tensor_scalar(
                    out=ou[:], in0=x_bf[:, 0:EMB],
                    scalar1=E_all[:, t, 4 * n:4 * n + 1], scalar2=None,
                    op0=Alu.mult)
                for j in (1, 2, 3):
                    nc.vector.tensor_scalar(
                        out=tmp[:], in0=x_bf[:, j * EMB:(j + 1) * EMB],
                        scalar1=E_all[:, t, 4 * n + j:4 * n + j + 1],
                        scalar2=None, op0=Alu.mult)
                    nc.vector.tensor_tensor(ou[:], ou[:], tmp[:], Alu.add)
                nc.vector.tensor_scalar(
                    out=tmp[:], in0=y_nb[:], scalar1=ww_all[:, t, n:n + 1],
                    scalar2=None, op0=Alu.mult)
                nc.vector.tensor_tensor(ou[:], ou[:], tmp[:], Alu.add)
                if CAST_DMA_OUT:
                    nc.gpsimd.dma_start(
                        out_ext[t * 128:(t + 1) * 128, n, :], ou[:])
                else:
                    of = ou_pool.tile([128, EMB], dt.float32, tag="of")
                    nc.vector.tensor_copy(of[:], ou[:])
                    eng = nc.scalar if n % 2 == 0 else nc.sync
                    eng.dma_start(
                        out_ext[t * 128:(t + 1) * 128, n, :], of[:])

        # ---- schedule ----
        for i in range(TPB):
            p1_tile(i)
        proj_block(0)
        for g in range(NBLK):
            p4_smalls(g)
            for i in range(TPB):
                if g + 1 < NBLK:
                    p1_tile((g + 1) * TPB + i)
                p567_tile(g * TPB + i)
            if g + 1 < NBLK:
                proj_block(g + 1)

    nc.compile()
    return nc


def _prep_weights(inputs):
    W_conv = np.asarray(inputs["W_conv"], np.float32)
    W_diss = np.asarray(inputs["W_diss"], np.float32)
    W_dtc = np.asarray(inputs["W_dtc"], np.float32)
    W_dtd = np.asarray(inputs["W_dtd"], np.float32)
    W_read = np.asarray(inputs["W_read"], np.float32)
    W_write = np.asarray(inputs["W_write"], np.float32)
    W_mod = np.asarray(inputs["W_mod"], np.float32)

    Wcat = np.concatenate([W_conv, W_diss, W_dtc, W_dtd, W_read, W_write],
                          axis=0)
    assert Wcat.shape == (NPROJ, IN_DIM)
    wcatT = np.ascontiguousarray(
        Wcat.T.reshape(IN_DIM // 128, 128, NPROJ).transpose(1, 0, 2)
    ).astype(BF16)
    wmodT = np.ascontiguousarray(
        W_mod.T.reshape(16, 128, EMB).transpose(1, 0, 2)
    ).astype(ml_dtypes.float8_e4m3)

    scal = dict(
        bias_c=float(np.asarray(inputs["log_dt_c"]).reshape(-1)[0]
                     + np.asarray(inputs["b_dtc"]).reshape(-1)[0]),
        bias_d=float(np.asarray(inputs["log_dt_d"]).reshape(-1)[0]
                     + np.asarray(inputs["b_dtd"]).reshape(-1)[0]),
        alpha_r=float(np.asarray(inputs["alpha_read_in"]).reshape(-1)[0]),
        alpha_w=float(np.asarray(inputs["alpha_write_out"]).reshape(-1)[0]),
    )

    cM = np.asarray(inputs["conserv_A"], np.float32) + \
        np.asarray(inputs["b_conv"], np.float32).reshape(NS, NS)
    skew_const = (cM - cM.T).reshape(-1)
    dissC = (np.asarray(inputs["diss_A"], np.float32) +
             np.asarray(inputs["b_diss"], np.float32).reshape(NS, NS)
             ).reshape(-1)
    eye16 = np.eye(NS, dtype=np.float32).reshape(-1)
    readin = np.asarray(inputs["read_in"], np.float32).reshape(-1)
    writeout = np.asarray(inputs["write_out"], np.float32).reshape(-1)
    cpack = np.concatenate([
        skew_const, dissC, eye16, readin, writeout,
        np.array([scal["bias_c"], scal["bias_d"]], np.float32)]
    ).astype(np.float32)
    assert cpack.shape == (58,)
    return wcatT, wmodT, cpack, scal


_NC_CACHE = {}


def kernel(**inputs):
    from concourse.bass_utils import run_bass_kernel_spmd

    x = np.asarray(inputs["x"], np.float32)
    B = x.shape[0]
    B_loc = B // N_CORES
    wcatT, wmodT, cpack, scal = _prep_weights(inputs)

    key = (B_loc, tuple(sorted(scal.items())))
    if key not in _NC_CACHE:
        _NC_CACHE[key] = _build(B_loc, scal)
    nc = _NC_CACHE[key]

    xf = x.reshape(B, IN_DIM)
    in_maps = []
    for i in range(N_CORES):
        in_maps.append({
            "x": np.ascontiguousarray(xf[i * B_loc:(i + 1) * B_loc]),
            "wcatT": wcatT,
            "wmodT": wmodT,
            "cpack": cpack,
        })

    trace = os.environ.get("KERNEL_TRACE", "0") == "1"
    res = run_bass_kernel_spmd(nc, in_maps, core_ids=list(range(N_CORES)),
                               trace=trace)
    if trace and res.exec_time_ns is not None:
        print(f"HW exec time: {res.exec_time_ns} ns")
        kernel.last_exec_time_ns = res.exec_time_ns
    out = np.concatenate([res.results[i]["out"] for i in range(N_CORES)],
                         axis=0)
    return out
